# revision 30
# baseline (speedup 1.0000x reference)
"""Trainium2 Bass kernel for nn_AutoRegressiveDistribution (MADE sampling).

Self-contained: hardcodes shapes/sharding. Shards batch B across 8 cores,
runs the D-step autoregressive sampling loop fully on-device per core.

Per-core structure (v2): FOUR independent chains, one per sample s
(width = BS = 128 batch rows each). Per chain and step i the critical
path is:

  hist-MM (PE, float32r, N=256 via free-dim duplication -> 1 cyc/row)
    -> relu (DVE, psum->sbuf, bf16-free)
    -> pair-MM (PE, N=2: only the (mu_i, ps_i) column pair, Wout columns
       are pair-interleaved host-side so the pair is contiguous)
    -> Exp -> Ln(bias=1) (Act, psum->psum softplus)
    -> fused FMA z = eps*sc + mu (GPSIMD scalar_tensor_tensor)
    -> full-state transpose (PE) -> single-row psum->sbuf copy (GPSIMD)
    -> next hist-MM

  The bulk contribution of block i to future column pairs [2i+2, 128)
  is emitted AFTER the FMA so it stays off the critical path; the
  ctx+bias base for the next block is preloaded into PSUM by a
  shifted-identity matmul (also f32r/dup). The loop-invariant
  a_base = Wc @ ctx + b1 is computed chunk-by-chunk interleaved with
  the first ~16 steps so it never stalls the chain.

  float32r bitcasts keep full fp32 precision at 1 cycle/row (vs 4 for
  fp32) for every z-path matmul; only PE-stationary operands and the
  tiny N<=2 pair matmuls stay plain fp32.
"""

import numpy as np
from contextlib import ExitStack

import concourse.bass as bass
import concourse.tile as tile
from concourse import bacc, mybir
from concourse.bass_utils import run_bass_kernel_spmd

D, H, CTX, B, S = 64, 1024, 256, 1024, 4
NCORES = 8
BS = B // NCORES          # 128 batch rows per core
K = S                     # 4 chains per core, one per sample
HP = 2048                 # padded hidden units: block i at [32*(i-1), +cnt[i])
NCH = HP // 128           # a_base unit chunks

FP32 = mybir.dt.float32
F32R = mybir.dt.float32r
BF16 = mybir.dt.bfloat16


def _made_struct():
    mh = (np.arange(H) % (D - 1)) + 1            # degrees 1..63
    perm = np.argsort(mh, kind="stable")
    mh_s = mh[perm]
    cnt = np.bincount(mh_s, minlength=D)          # cnt[d] = #units of degree d
    off = np.concatenate([[0], np.cumsum(cnt)[:-1]]).astype(np.int64)
    return mh, perm, mh_s, cnt, off


def _prep_weights(W1, b1, Wc, Wout):
    """Mask + permute + 32-pad weights host-side (cheap, O(weight size))."""
    mh, perm, mh_s, cnt, off = _made_struct()
    m0 = np.arange(1, D + 1)
    M1 = (mh[:, None] >= m0[None, :]).astype(np.float32)          # (H, D)
    mout = np.concatenate([m0, m0])                                # (2D,)
    Mout = (mout[:, None] > mh[None, :]).astype(np.float32)        # (2D, H)
    W1m = (W1 * M1)[perm]                   # (H, D) permuted rows
    Woutm = (Wout * Mout)[:, perm]          # (2D, H) permuted cols
    src = np.arange(H)
    pdst = 32 * (mh_s - 1) + (src - off[mh_s])   # padded slot of sorted unit
    W1T = np.zeros((D, HP), np.float32)
    W1T[:, pdst] = W1m.T
    WcT = np.zeros((CTX, HP), np.float32)
    WcT[:, pdst] = Wc[perm].T
    b1c = np.zeros((128, NCH), np.float32)
    b1p = np.zeros((HP,), np.float32)
    b1p[pdst] = b1[perm]
    b1c[:, :] = b1p.reshape(NCH, 128).T
    # pair-interleaved output weights: col 2j = mu_j, col 2j+1 = prescale_j
    WoutP = np.zeros((32, D - 1, 2 * D), np.float32)
    mu_rows = Woutm[:D, :]      # (D, H)
    ps_rows = Woutm[D:, :]      # (D, H)
    for j in range(D):
        WoutP[pdst % 32, (mh_s - 1), 2 * j] = mu_rows[j, src]
        WoutP[pdst % 32, (mh_s - 1), 2 * j + 1] = ps_rows[j, src]
    return W1T, WoutP, WcT, b1c


def _prep_bout(bout):
    boutP = np.zeros((128, 2 * D), np.float32)
    boutP[:, 0::2] = bout[:D][None, :]
    boutP[:, 1::2] = bout[D:][None, :]
    return boutP


_PROGRAM_CACHE = None


def _pin_act_table():
    """Make Exp/Ln/Relu resolvable only via natural_log_exp_and_others so
    the act-table chooser doesn't thrash (each LoadActFuncSet ~1.3us)."""
    import concourse.bacc as bacc_mod
    from concourse import hw_specs
    orig = hw_specs.get_activation_tables
    AF = mybir.ActivationFunctionType
    pin = {AF.Exp, AF.Ln, AF.Relu}

    def filtered(arch):
        out = {}
        for name, fns in orig(arch).items():
            if name == "natural_log_exp_and_others":
                out[name] = set(fns)
            else:
                out[name] = set(fns) - pin
        return out

    bacc_mod.get_activation_tables = filtered


def _dup(ap):
    """Duplicate an AP along a broadcast free dim (doubles free size so
    float32r matmuls hit N>=256 and run at 1 cycle/row)."""
    return bass.AP(ap.tensor, ap.offset, [ap.ap[0], [0, 2], ap.ap[-1]])


def _build_program():
    global _PROGRAM_CACHE
    if _PROGRAM_CACHE is not None:
        return _PROGRAM_CACHE
    _pin_act_table()
    _, _, mh_s, cnt, off = _made_struct()

    nc = bacc.Bacc("TRN2", target_bir_lowering=False, debug=False,
                   num_devices=NCORES)

    ctx_d = nc.dram_tensor("ctx", (BS, CTX), FP32, kind="ExternalInput")
    eps_d = nc.dram_tensor("eps", (S, BS, D), FP32, kind="ExternalInput")
    w1t_d = nc.dram_tensor("w1t", (D, HP), F32R, kind="ExternalInput")
    woutpb_d = nc.dram_tensor("woutpb", (32, D - 1, 2 * D), BF16,
                              kind="ExternalInput")
    wct_d = nc.dram_tensor("wct", (CTX, HP), F32R, kind="ExternalInput")
    b1c_d = nc.dram_tensor("b1c", (128, NCH), FP32, kind="ExternalInput")
    boutp_d = nc.dram_tensor("boutp", (128, 2 * D), FP32, kind="ExternalInput")
    boutpb_d = nc.dram_tensor("boutpb", (128, 2 * D), BF16,
                              kind="ExternalInput")
    ident_d = nc.dram_tensor("ident", (128, 128), F32R, kind="ExternalInput")
    z_d = nc.dram_tensor("z_out", (S, BS, D), F32R, kind="ExternalOutput")
    mu_d = nc.dram_tensor("mu_out", (S, BS, D), FP32, kind="ExternalOutput")
    sc_d = nc.dram_tensor("sc_out", (S, BS, D), FP32, kind="ExternalOutput")

    AF = mybir.ActivationFunctionType
    OP = mybir.AluOpType

    with tile.TileContext(nc) as tc, ExitStack() as ctx:
        singles = ctx.enter_context(tc.tile_pool(name="singles", bufs=1))
        abp = ctx.enter_context(tc.tile_pool(name="abp", bufs=2))
        psum = ctx.enter_context(tc.tile_pool(name="psum", bufs=1,
                                              space="PSUM"))

        # ---- input DMAs, priority order ----
        ctx_sb = singles.tile([BS, CTX], FP32)
        nc.sync.dma_start(ctx_sb[:], ctx_d.ap())
        eps_sb = singles.tile([BS, S, D], FP32)
        nc.sync.dma_start(eps_sb[:], eps_d.ap().rearrange("s b d -> b s d"))
        boutp_sb = singles.tile([128, 2 * D], FP32)
        nc.sync.dma_start(boutp_sb[:], boutp_d.ap())
        boutpb_sb = singles.tile([128, 2 * D], BF16)
        nc.sync.dma_start(boutpb_sb[:], boutpb_d.ap())
        ident_sb = singles.tile([128, 128], F32R)
        nc.sync.dma_start(ident_sb[:], ident_d.ap())
        b1c_sb = singles.tile([128, NCH], FP32)
        nc.sync.dma_start(b1c_sb[:], b1c_d.ap())
        wct_sb = singles.tile([128, 2, HP], F32R)
        w1t_sb = singles.tile([D, HP], F32R)
        woutpb_sb = singles.tile([32, D - 1, 2 * D], BF16)
        QH = HP // 4
        for q in range(4):
            nc.sync.dma_start(
                wct_sb[:, :, q * QH:(q + 1) * QH],
                wct_d.ap()[:, q * QH:(q + 1) * QH]
                .rearrange("(k p) h -> p k h", p=128))
            nc.sync.dma_start(w1t_sb[:, q * QH:(q + 1) * QH],
                              w1t_d.ap()[:, q * QH:(q + 1) * QH])
            b0, b1_ = [(0, 16), (16, 32), (32, 48), (48, 63)][q]
            nc.sync.dma_start(woutpb_sb[:, b0:b1_, :],
                              woutpb_d.ap()[:, b0:b1_, :])

        onesb_sb = singles.tile([1, 128], BF16)
        nc.vector.memset(onesb_sb[:], 1.0)


        # ---- PSUM layout, shaped by the HW rule that a start=True matmul
        # marks its whole 2KB bank pending-zero (so a bank can host only one
        # accumulation lifetime at a time):
        #  bank tOUT: all 4 persistent OUT accumulators, seeded by ONE
        #             start=True matmul, then only start=False forever.
        #  bank tZT:  all 4 transpose targets (every write is a fresh
        #             single-matmul start=True group -> safe to share).
        #  banks tPA[c]: per-chain psA (ident start=True -> hist stop,
        #             WAW-ordered, nothing else matmuls this bank) plus the
        #             Act-written scPS strip (engines ignore pending flags).
        #  bank tSET: setup scratch; only single-matmul start=True groups.
        tOUT = psum.tile([128, K, 2 * D], FP32, name="tOUT")
        tZT = psum.tile([D, K, BS], F32R, name="tZT")
        tPA = [psum.tile([128, 512], FP32, tag=f"tPA{c}", name=f"tPA{c}")
               for c in range(K)]
        tSET = psum.tile([128, 512], FP32, name="tSET")
        outP = [tOUT[:, c, :] for c in range(K)]              # (128, 128)
        zTps = [tZT[:, c, :] for c in range(K)]               # (64, 128)
        SC0 = 256                                             # scPS base col

        def aps_ap(c, nn):
            """psA view of tPA[c]: (nn, 2, 128) at cols 0:256."""
            t = tPA[c][0:nn, 0:256]
            return bass.AP(t.tensor, t.offset, [t.ap[0], [128, 2], [1, 128]])

        # ---- ctxT: (BS, CTX) -> (128, 2, BS) via 2 PE transposes ----
        ctxT_sb = singles.tile([128, 2, BS], F32R)
        for kk in range(2):
            ps = tSET[:, kk * BS:kk * BS + BS]
            nc.tensor.transpose(ps, ctx_sb[:, kk * 128:(kk + 1) * 128],
                                ident_sb[:].bitcast(FP32))
            nc.vector.tensor_copy(ctxT_sb[:, kk, :], ps)

        # ---- per-chain state ----
        a_base = singles.tile([128, NCH, 128], F32R)
        z2 = [singles.tile([BS, D], F32R, tag=f"z{c}", name=f"z{c}")
              for c in range(K)]
        muA = singles.tile([BS, K, D], FP32)
        scA = singles.tile([BS, K, D], FP32)
        zT = [singles.tile([D, BS], F32R, tag=f"zT{c}", name=f"zT{c}")
              for c in range(K)]

        for c in range(K):
            nc.vector.memset(z2[c][:].bitcast(FP32), 0.0)

        def a_base_chunk(cc):
            """a_base[:, cc, :] = (WcT chunk).T @ ctxT + b1 chunk.

            In-place accumulation in the tSET bank is safe: every matmul
            writer of this bank is ordered by WAW or data deps, so no
            start=True interloper can land between the two halves."""
            ps = tSET[:, 256:384]
            for kk in range(2):
                nc.tensor.matmul(
                    ps,
                    wct_sb[:, kk, cc * 128:(cc + 1) * 128],
                    ctxT_sb[:, kk, :],
                    start=(kk == 0), stop=(kk == 1))
            nc.vector.tensor_scalar_add(a_base[:, cc, :], ps,
                                        b1c_sb[:, cc:cc + 1])

        a_base_chunk(0)
        a_base_chunk(1)

        def bridge_t(c):
            nc.tensor.transpose(zTps[c], z2[c][:], ident_sb[:])

        def bridge_c(c, i):
            g = 32 * (i // 32)
            src_rows = tZT[g:g + 32, c, :]
            if c < 2:
                nc.vector.tensor_copy(zT[c][g:g + 32, :], src_rows)
            else:
                nc.scalar.copy(zT[c][g:g + 32, :], src_rows)

        def bridge(c, i):
            """z2[c] -> zT[c] row i: full-state PE transpose + 1-row copy."""
            bridge_t(c)
            bridge_c(c, i)

        # ---- step 0: bias-only ----
        # one seed matmul covers all 4 OUT accumulators (single start=True
        # lifetime for the whole bank)
        br = boutpb_sb[0:1, :]
        br4 = bass.AP(br.tensor, br.offset, [br.ap[0], [0, K], br.ap[-1]])
        nc.tensor.matmul(tOUT[:, :, :], onesb_sb[:], br4,
                         start=True, stop=False, skip_group_check=True)
        for c in range(K):
            sp = tPA[c][:, SC0 + D:SC0 + D + 1]
            nc.scalar.activation(out=sp, in_=boutp_sb[:, 1:2],
                                 func=AF.Exp, bias=0.0, scale=1.0)
            nc.scalar.activation(out=tPA[c][:, SC0:SC0 + 1], in_=sp,
                                 func=AF.Ln, bias=1.0, scale=1.0)
            nc.vector.scalar_tensor_tensor(
                out=z2[c][:, 0:1], in0=eps_sb[:, c, 0:1],
                scalar=tPA[c][:, SC0:SC0 + 1], in1=boutp_sb[:, 0:1],
                op0=OP.mult, op1=OP.add)
            bridge(c, 0)

        # ---- steps 1..63 ----
        for i in range(1, D):
            nn = int(cnt[i])
            pp = 32 * (i - 1)
            cc, pl = pp // 128, pp % 128
            kk = pl + nn                  # ident rows anchored at 0 so both
                                          # group matmuls share tile pos (0,0)
            if i >= 5 and (i - 5) % 4 == 0:
                nxt = 2 + (i - 5) // 4
                if nxt < NCH:
                    a_base_chunk(nxt)
            # chain-major emission; the tile scheduler handles interleaving.
            # bridge(c) is emitted one chain-block late so copies don't block
            # the next chain's softplus in the Act/DVE queues.
            pending = []
            for c in range(K):
                aps = aps_ap(c, nn)
                nc.tensor.matmul(
                    aps,
                    ident_sb[0:kk, pl:pl + nn],
                    _dup(a_base[0:kk, cc, :]),
                    start=True, stop=False)
                nc.tensor.matmul(
                    aps,
                    w1t_sb[0:i, pp:pp + nn],
                    _dup(zT[c][0:i, :]),
                    start=False, stop=True)
                if pending:
                    bridge_t(pending[0])
                ab = abp.tile([nn, 128], BF16, tag=f"ab{c}")
                nc.vector.tensor_scalar_max(ab[:], tPA[c][0:nn, 0:128], 0.0)
                nc.tensor.matmul(tOUT[:, c, 2 * i:2 * i + 2],
                                 ab[:],
                                 woutpb_sb[0:nn, i - 1, 2 * i:2 * i + 2],
                                 start=False, stop=(i == D - 1),
                                 skip_group_check=True)
                if i < D - 1:
                    nc.tensor.matmul(tOUT[:, c, 2 * i + 2:2 * D],
                                     ab[:],
                                     woutpb_sb[0:nn, i - 1, 2 * i + 2:],
                                     start=False, stop=False,
                                     skip_group_check=True)
                sp = tPA[c][:, SC0 + D + (i % 2):SC0 + D + (i % 2) + 1]
                nc.scalar.activation(out=sp,
                                     in_=tOUT[:, c, 2 * i + 1:2 * i + 2],
                                     func=AF.Exp, bias=0.0, scale=1.0)
                nc.scalar.activation(out=tPA[c][:, SC0 + i:SC0 + i + 1],
                                     in_=sp, func=AF.Ln, bias=1.0, scale=1.0)
                nc.vector.scalar_tensor_tensor(
                    out=z2[c][:, i:i + 1], in0=eps_sb[:, c, i:i + 1],
                    scalar=tPA[c][:, SC0 + i:SC0 + i + 1],
                    in1=tOUT[:, c, 2 * i:2 * i + 1],
                    op0=OP.mult, op1=OP.add)
                if pending:
                    bridge_c(pending.pop(), i)
                if i < D - 1:
                    pending.append(c)

            if pending:
                bridge(pending.pop(), i)
            # progressive output drain: first halves are final after step 32
            if i == 34:
                for c in range(K):
                    sl = tOUT[:, c, 0:D]
                    mu_ap = bass.AP(sl.tensor, sl.offset, [sl.ap[0], [2, 32]])
                    nc.vector.tensor_copy(muA[:, c, 0:32], mu_ap)
                    nc.scalar.copy(scA[:, c, 0:32], tPA[c][:, SC0:SC0 + 32])
                    nc.sync.dma_start(z_d.ap()[c][:, 0:32], z2[c][:, 0:32])
                nc.sync.dma_start(
                    mu_d.ap()[:, :, 0:32].rearrange("s b d -> b s d"),
                    muA[:, :, 0:32])
                nc.sync.dma_start(
                    sc_d.ap()[:, :, 0:32].rearrange("s b d -> b s d"),
                    scA[:, :, 0:32])

        # ---- tail: extract mu (even cols) + sc, DMA out ----
        for c in range(K):
            sl = tOUT[:, c, D:2 * D]
            mu_ap = bass.AP(sl.tensor, sl.offset, [sl.ap[0], [2, 32]])
            nc.vector.tensor_copy(muA[:, c, 32:64], mu_ap)
            nc.scalar.copy(scA[:, c, 32:64], tPA[c][:, SC0 + 32:SC0 + D])
            nc.sync.dma_start(z_d.ap()[c][:, 32:64], z2[c][:, 32:64])
        nc.sync.dma_start(
            mu_d.ap()[:, :, 32:64].rearrange("s b d -> b s d"),
            muA[:, :, 32:64])
        nc.sync.dma_start(
            sc_d.ap()[:, :, 32:64].rearrange("s b d -> b s d"),
            scA[:, :, 32:64])

    nc.compile()
    _PROGRAM_CACHE = nc
    return nc


def _in_maps(context, eps, W1, b1, Wc, Wout, bout):
    import ml_dtypes
    W1T, WoutP, WcT, b1c = _prep_weights(W1, b1, Wc, Wout)
    WoutPb = WoutP.astype(ml_dtypes.bfloat16)
    boutP = _prep_bout(bout)
    ident = np.eye(128, dtype=np.float32)
    maps = []
    for c in range(NCORES):
        maps.append({
            "ctx": np.ascontiguousarray(context[c * BS:(c + 1) * BS]),
            "eps": np.ascontiguousarray(eps[:, c * BS:(c + 1) * BS]),
            "w1t": W1T, "woutpb": WoutPb, "wct": WcT, "b1c": b1c,
            "boutp": boutP, "boutpb": boutP.astype(ml_dtypes.bfloat16),
            "ident": ident,
        })
    return maps


def run(context, eps, W1, b1, Wc, Wout, bout, trace=False):
    context = np.asarray(context, np.float32)
    eps = np.asarray(eps, np.float32)
    W1 = np.asarray(W1, np.float32)
    b1 = np.asarray(b1, np.float32)
    Wc = np.asarray(Wc, np.float32)
    Wout = np.asarray(Wout, np.float32)
    bout = np.asarray(bout, np.float32)
    nc = _build_program()
    maps = _in_maps(context, eps, W1, b1, Wc, Wout, bout)
    res = run_bass_kernel_spmd(nc, maps, core_ids=list(range(NCORES)),
                               trace=trace)
    z = np.empty((S, B, D), np.float32)
    mu = np.empty((S, B, D), np.float32)
    sc = np.empty((S, B, D), np.float32)
    for c in range(NCORES):
        z[:, c * BS:(c + 1) * BS] = res.results[c]["z_out"]
        mu[:, c * BS:(c + 1) * BS] = res.results[c]["mu_out"]
        sc[:, c * BS:(c + 1) * BS] = res.results[c]["sc_out"]
    return (z, mu, sc), res


def kernel(context, eps, W1, b1, Wc, Wout, bout):
    (z, mu, sc), _ = run(context, eps, W1, b1, Wc, Wout, bout)
    return z, mu, sc


# revision 31
# speedup vs baseline: 1.0677x; 1.0677x over previous
"""Trainium2 Bass kernel for nn_AutoRegressiveDistribution (MADE sampling).

Self-contained: hardcodes shapes/sharding. Shards batch B across 8 cores,
runs the D-step autoregressive sampling loop fully on-device per core.

Per-core structure (v2): FOUR independent chains, one per sample s
(width = BS = 128 batch rows each). Per chain and step i the critical
path is:

  hist-MM (PE, float32r, N=256 via free-dim duplication -> 1 cyc/row)
    -> relu (DVE, psum->sbuf, bf16-free)
    -> pair-MM (PE, N=2: only the (mu_i, ps_i) column pair, Wout columns
       are pair-interleaved host-side so the pair is contiguous)
    -> Exp -> Ln(bias=1) (Act, psum->psum softplus)
    -> fused FMA z = eps*sc + mu (GPSIMD scalar_tensor_tensor)
    -> full-state transpose (PE) -> single-row psum->sbuf copy (GPSIMD)
    -> next hist-MM

  The bulk contribution of block i to future column pairs [2i+2, 128)
  is emitted AFTER the FMA so it stays off the critical path; the
  ctx+bias base for the next block is preloaded into PSUM by a
  shifted-identity matmul (also f32r/dup). The loop-invariant
  a_base = Wc @ ctx + b1 is computed chunk-by-chunk interleaved with
  the first ~16 steps so it never stalls the chain.

  float32r bitcasts keep full fp32 precision at 1 cycle/row (vs 4 for
  fp32) for every z-path matmul; only PE-stationary operands and the
  tiny N<=2 pair matmuls stay plain fp32.
"""

import numpy as np
from contextlib import ExitStack

import concourse.bass as bass
import concourse.tile as tile
from concourse import bacc, mybir
from concourse.bass_utils import run_bass_kernel_spmd

D, H, CTX, B, S = 64, 1024, 256, 1024, 4
NCORES = 8
BS = B // NCORES          # 128 batch rows per core
K = S                     # 4 chains per core, one per sample
HP = 2048                 # padded hidden units: block i at [32*(i-1), +cnt[i])
NCH = HP // 128           # a_base unit chunks

FP32 = mybir.dt.float32
F32R = mybir.dt.float32r
BF16 = mybir.dt.bfloat16


def _made_struct():
    mh = (np.arange(H) % (D - 1)) + 1            # degrees 1..63
    perm = np.argsort(mh, kind="stable")
    mh_s = mh[perm]
    cnt = np.bincount(mh_s, minlength=D)          # cnt[d] = #units of degree d
    off = np.concatenate([[0], np.cumsum(cnt)[:-1]]).astype(np.int64)
    return mh, perm, mh_s, cnt, off


def _prep_weights(W1, b1, Wc, Wout):
    """Mask + permute + 32-pad weights host-side (cheap, O(weight size))."""
    mh, perm, mh_s, cnt, off = _made_struct()
    m0 = np.arange(1, D + 1)
    M1 = (mh[:, None] >= m0[None, :]).astype(np.float32)          # (H, D)
    mout = np.concatenate([m0, m0])                                # (2D,)
    Mout = (mout[:, None] > mh[None, :]).astype(np.float32)        # (2D, H)
    W1m = (W1 * M1)[perm]                   # (H, D) permuted rows
    Woutm = (Wout * Mout)[:, perm]          # (2D, H) permuted cols
    src = np.arange(H)
    pdst = 32 * (mh_s - 1) + (src - off[mh_s])   # padded slot of sorted unit
    W1T = np.zeros((D, HP), np.float32)
    W1T[:, pdst] = W1m.T
    WcT = np.zeros((CTX, HP), np.float32)
    WcT[:, pdst] = Wc[perm].T
    b1c = np.zeros((128, NCH), np.float32)
    b1p = np.zeros((HP,), np.float32)
    b1p[pdst] = b1[perm]
    b1c[:, :] = b1p.reshape(NCH, 128).T
    # pair-interleaved output weights: col 2j = mu_j, col 2j+1 = prescale_j
    WoutP = np.zeros((32, D - 1, 2 * D), np.float32)
    mu_rows = Woutm[:D, :]      # (D, H)
    ps_rows = Woutm[D:, :]      # (D, H)
    for j in range(D):
        WoutP[pdst % 32, (mh_s - 1), 2 * j] = mu_rows[j, src]
        WoutP[pdst % 32, (mh_s - 1), 2 * j + 1] = ps_rows[j, src]
    return W1T, WoutP, WcT, b1c


def _prep_bout(bout):
    boutP = np.zeros((128, 2 * D), np.float32)
    boutP[:, 0::2] = bout[:D][None, :]
    boutP[:, 1::2] = bout[D:][None, :]
    return boutP


_PROGRAM_CACHE = None


def _pin_act_table():
    """Make Exp/Ln/Relu resolvable only via natural_log_exp_and_others so
    the act-table chooser doesn't thrash (each LoadActFuncSet ~1.3us)."""
    import concourse.bacc as bacc_mod
    from concourse import hw_specs
    orig = hw_specs.get_activation_tables
    AF = mybir.ActivationFunctionType
    pin = {AF.Exp, AF.Ln, AF.Relu}

    def filtered(arch):
        out = {}
        for name, fns in orig(arch).items():
            if name == "natural_log_exp_and_others":
                out[name] = set(fns)
            else:
                out[name] = set(fns) - pin
        return out

    bacc_mod.get_activation_tables = filtered


def _dup(ap):
    """Duplicate an AP along a broadcast free dim (doubles free size so
    float32r matmuls hit N>=256 and run at 1 cycle/row)."""
    return bass.AP(ap.tensor, ap.offset, [ap.ap[0], [0, 2], ap.ap[-1]])


def _build_program():
    global _PROGRAM_CACHE
    if _PROGRAM_CACHE is not None:
        return _PROGRAM_CACHE
    _pin_act_table()
    _, _, mh_s, cnt, off = _made_struct()

    nc = bacc.Bacc("TRN2", target_bir_lowering=False, debug=False,
                   num_devices=NCORES)

    ctx_d = nc.dram_tensor("ctx", (BS, CTX), FP32, kind="ExternalInput")
    eps_d = nc.dram_tensor("eps", (S, BS, D), FP32, kind="ExternalInput")
    w1t_d = nc.dram_tensor("w1t", (D, HP), F32R, kind="ExternalInput")
    woutpb_d = nc.dram_tensor("woutpb", (32, D - 1, 2 * D), BF16,
                              kind="ExternalInput")
    wct_d = nc.dram_tensor("wct", (CTX, HP), F32R, kind="ExternalInput")
    b1c_d = nc.dram_tensor("b1c", (128, NCH), FP32, kind="ExternalInput")
    boutp_d = nc.dram_tensor("boutp", (128, 2 * D), FP32, kind="ExternalInput")
    boutpb_d = nc.dram_tensor("boutpb", (128, 2 * D), BF16,
                              kind="ExternalInput")
    ident_d = nc.dram_tensor("ident", (128, 128), F32R, kind="ExternalInput")
    z_d = nc.dram_tensor("z_out", (S, BS, D), F32R, kind="ExternalOutput")
    mu_d = nc.dram_tensor("mu_out", (S, BS, D), FP32, kind="ExternalOutput")
    sc_d = nc.dram_tensor("sc_out", (S, BS, D), FP32, kind="ExternalOutput")

    AF = mybir.ActivationFunctionType
    OP = mybir.AluOpType

    with tile.TileContext(nc) as tc, ExitStack() as ctx:
        singles = ctx.enter_context(tc.tile_pool(name="singles", bufs=1))
        abp = ctx.enter_context(tc.tile_pool(name="abp", bufs=2))
        psum = ctx.enter_context(tc.tile_pool(name="psum", bufs=1,
                                              space="PSUM"))

        # ---- input DMAs, priority order ----
        ctx_sb = singles.tile([BS, CTX], FP32)
        nc.sync.dma_start(ctx_sb[:], ctx_d.ap())
        eps_sb = singles.tile([BS, S, D], FP32)
        nc.sync.dma_start(eps_sb[:], eps_d.ap().rearrange("s b d -> b s d"))
        boutp_sb = singles.tile([128, 2 * D], FP32)
        nc.sync.dma_start(boutp_sb[:], boutp_d.ap())
        boutpb_sb = singles.tile([128, 2 * D], BF16)
        nc.sync.dma_start(boutpb_sb[:], boutpb_d.ap())
        ident_sb = singles.tile([128, 128], F32R)
        nc.sync.dma_start(ident_sb[:], ident_d.ap())
        b1c_sb = singles.tile([128, NCH], FP32)
        nc.sync.dma_start(b1c_sb[:], b1c_d.ap())
        wct_sb = singles.tile([128, 2, HP], F32R)
        w1t_sb = singles.tile([D, HP], F32R)
        woutpb_sb = singles.tile([32, D - 1, 2 * D], BF16)
        QH = HP // 4
        for q in range(4):
            nc.sync.dma_start(
                wct_sb[:, :, q * QH:(q + 1) * QH],
                wct_d.ap()[:, q * QH:(q + 1) * QH]
                .rearrange("(k p) h -> p k h", p=128))
            nc.sync.dma_start(w1t_sb[:, q * QH:(q + 1) * QH],
                              w1t_d.ap()[:, q * QH:(q + 1) * QH])
            b0, b1_ = [(0, 16), (16, 32), (32, 48), (48, 63)][q]
            nc.sync.dma_start(woutpb_sb[:, b0:b1_, :],
                              woutpb_d.ap()[:, b0:b1_, :])

        onesb_sb = singles.tile([1, 128], BF16)
        nc.vector.memset(onesb_sb[:], 1.0)


        # ---- PSUM layout, shaped by the HW rule that a start=True matmul
        # marks its whole 2KB bank pending-zero (so a bank can host only one
        # accumulation lifetime at a time):
        #  bank tOUT: all 4 persistent OUT accumulators, seeded by ONE
        #             start=True matmul, then only start=False forever.
        #  bank tZT:  all 4 transpose targets (every write is a fresh
        #             single-matmul start=True group -> safe to share).
        #  banks tPA[c]: per-chain psA (ident start=True -> hist stop,
        #             WAW-ordered, nothing else matmuls this bank) plus the
        #             Act-written scPS strip (engines ignore pending flags).
        #  bank tSET: setup scratch; only single-matmul start=True groups.
        tOUT = psum.tile([128, K, 2 * D], FP32, name="tOUT")
        tZT = psum.tile([D, K, BS], F32R, name="tZT")
        tPA = [psum.tile([128, 512], FP32, tag=f"tPA{c}", name=f"tPA{c}")
               for c in range(K)]
        tSET = psum.tile([128, 512], FP32, name="tSET")
        outP = [tOUT[:, c, :] for c in range(K)]              # (128, 128)
        zTps = [tZT[:, c, :] for c in range(K)]               # (64, 128)
        SC0 = 256                                             # scPS base col

        def aps_ap(c, nn):
            """psA view of tPA[c]: (nn, 2, 128) at cols 0:256."""
            t = tPA[c][0:nn, 0:256]
            return bass.AP(t.tensor, t.offset, [t.ap[0], [128, 2], [1, 128]])

        # ---- ctxT: (BS, CTX) -> (128, 2, BS) via 2 PE transposes ----
        ctxT_sb = singles.tile([128, 2, BS], F32R)
        for kk in range(2):
            ps = tSET[:, kk * BS:kk * BS + BS]
            nc.tensor.transpose(ps, ctx_sb[:, kk * 128:(kk + 1) * 128],
                                ident_sb[:].bitcast(FP32))
            nc.vector.tensor_copy(ctxT_sb[:, kk, :], ps)

        # ---- per-chain state ----
        a_base = singles.tile([128, NCH, 128], F32R)
        z2 = [singles.tile([BS, D], F32R, tag=f"z{c}", name=f"z{c}")
              for c in range(K)]
        muA = singles.tile([BS, K, D], FP32)
        scA = singles.tile([BS, K, D], FP32)
        zT = [singles.tile([D, BS], F32R, tag=f"zT{c}", name=f"zT{c}")
              for c in range(K)]

        for c in range(K):
            nc.vector.memset(z2[c][:].bitcast(FP32), 0.0)

        def a_base_chunk(cc):
            """a_base[:, cc, :] = (WcT chunk).T @ ctxT + b1 chunk.

            In-place accumulation in the tSET bank is safe: every matmul
            writer of this bank is ordered by WAW or data deps, so no
            start=True interloper can land between the two halves."""
            ps = tSET[:, 256:384]
            for kk in range(2):
                nc.tensor.matmul(
                    ps,
                    wct_sb[:, kk, cc * 128:(cc + 1) * 128],
                    ctxT_sb[:, kk, :],
                    start=(kk == 0), stop=(kk == 1))
            nc.vector.tensor_scalar_add(a_base[:, cc, :], ps,
                                        b1c_sb[:, cc:cc + 1])

        a_base_chunk(0)
        a_base_chunk(1)

        def bridge_t(c):
            nc.tensor.transpose(zTps[c], z2[c][:], ident_sb[:])

        def bridge_c(c, i):
            g = 32 * (i // 32)
            src_rows = tZT[g:g + 32, c, :]
            if c < 2:
                nc.vector.tensor_copy(zT[c][g:g + 32, :], src_rows)
            else:
                nc.scalar.copy(zT[c][g:g + 32, :], src_rows)

        def bridge(c, i):
            """z2[c] -> zT[c] row i: full-state PE transpose + 1-row copy."""
            bridge_t(c)
            bridge_c(c, i)

        # ---- step 0: bias-only ----
        # one seed matmul covers all 4 OUT accumulators (single start=True
        # lifetime for the whole bank)
        br = boutpb_sb[0:1, :]
        br4 = bass.AP(br.tensor, br.offset, [br.ap[0], [0, K], br.ap[-1]])
        nc.tensor.matmul(tOUT[:, :, :], onesb_sb[:], br4,
                         start=True, stop=False, skip_group_check=True)
        for c in range(K):
            sp = tPA[c][:, SC0 + D:SC0 + D + 1]
            nc.scalar.activation(out=sp, in_=boutp_sb[:, 1:2],
                                 func=AF.Exp, bias=0.0, scale=1.0)
            nc.scalar.activation(out=tPA[c][:, SC0:SC0 + 1], in_=sp,
                                 func=AF.Ln, bias=1.0, scale=1.0)
            nc.vector.scalar_tensor_tensor(
                out=z2[c][:, 0:1], in0=eps_sb[:, c, 0:1],
                scalar=tPA[c][:, SC0:SC0 + 1], in1=boutp_sb[:, 0:1],
                op0=OP.mult, op1=OP.add)
            bridge(c, 0)

        # ---- steps 1..63 ----
        for i in range(1, D):
            nn = int(cnt[i])
            pp = 32 * (i - 1)
            cc, pl = pp // 128, pp % 128
            kk = pl + nn                  # ident rows anchored at 0 so both
                                          # group matmuls share tile pos (0,0)
            if i >= 5 and (i - 5) % 4 == 0:
                nxt = 2 + (i - 5) // 4
                if nxt < NCH:
                    a_base_chunk(nxt)
            # chain-major emission; the tile scheduler handles interleaving.
            # bridge(c) is emitted one chain-block late so copies don't block
            # the next chain's softplus in the Act/DVE queues.
            pending = []
            for c in range(K):
                aps = aps_ap(c, nn)
                nc.tensor.matmul(
                    aps,
                    ident_sb[0:kk, pl:pl + nn],
                    _dup(a_base[0:kk, cc, :]),
                    start=True, stop=False)
                nc.tensor.matmul(
                    aps,
                    w1t_sb[0:i, pp:pp + nn],
                    _dup(zT[c][0:i, :]),
                    start=False, stop=True)
                ab = abp.tile([nn, 128], BF16, tag=f"ab{c}")
                nc.vector.tensor_scalar_max(ab[:], tPA[c][0:nn, 0:128], 0.0)
                nc.tensor.matmul(tOUT[:, c, 2 * i:2 * i + 2],
                                 ab[:],
                                 woutpb_sb[0:nn, i - 1, 2 * i:2 * i + 2],
                                 start=False, stop=(i == D - 1),
                                 skip_group_check=True)
                if i < D - 1:
                    nc.tensor.matmul(tOUT[:, c, 2 * i + 2:2 * D],
                                     ab[:],
                                     woutpb_sb[0:nn, i - 1, 2 * i + 2:],
                                     start=False, stop=False,
                                     skip_group_check=True)
                sp = tPA[c][:, SC0 + D + (i % 2):SC0 + D + (i % 2) + 1]
                nc.scalar.activation(out=sp,
                                     in_=tOUT[:, c, 2 * i + 1:2 * i + 2],
                                     func=AF.Exp, bias=0.0, scale=1.0)
                nc.scalar.activation(out=tPA[c][:, SC0 + i:SC0 + i + 1],
                                     in_=sp, func=AF.Ln, bias=1.0, scale=1.0)
                nc.vector.scalar_tensor_tensor(
                    out=z2[c][:, i:i + 1], in0=eps_sb[:, c, i:i + 1],
                    scalar=tPA[c][:, SC0 + i:SC0 + i + 1],
                    in1=tOUT[:, c, 2 * i:2 * i + 1],
                    op0=OP.mult, op1=OP.add)
                if pending:
                    bridge(pending.pop(), i)
                if i < D - 1:
                    pending.append(c)

            if pending:
                bridge(pending.pop(), i)
            # progressive output drain: first halves are final after step 32
            if i == 34:
                for c in range(K):
                    sl = tOUT[:, c, 0:D]
                    mu_ap = bass.AP(sl.tensor, sl.offset, [sl.ap[0], [2, 32]])
                    nc.vector.tensor_copy(muA[:, c, 0:32], mu_ap)
                    nc.scalar.copy(scA[:, c, 0:32], tPA[c][:, SC0:SC0 + 32])
                    nc.sync.dma_start(z_d.ap()[c][:, 0:32], z2[c][:, 0:32])
                nc.sync.dma_start(
                    mu_d.ap()[:, :, 0:32].rearrange("s b d -> b s d"),
                    muA[:, :, 0:32])
                nc.sync.dma_start(
                    sc_d.ap()[:, :, 0:32].rearrange("s b d -> b s d"),
                    scA[:, :, 0:32])

        # ---- tail: extract mu (even cols) + sc, DMA out ----
        for c in range(K):
            sl = tOUT[:, c, D:2 * D]
            mu_ap = bass.AP(sl.tensor, sl.offset, [sl.ap[0], [2, 32]])
            nc.vector.tensor_copy(muA[:, c, 32:64], mu_ap)
            nc.scalar.copy(scA[:, c, 32:64], tPA[c][:, SC0 + 32:SC0 + D])
            nc.sync.dma_start(z_d.ap()[c][:, 32:64], z2[c][:, 32:64])
        nc.sync.dma_start(
            mu_d.ap()[:, :, 32:64].rearrange("s b d -> b s d"),
            muA[:, :, 32:64])
        nc.sync.dma_start(
            sc_d.ap()[:, :, 32:64].rearrange("s b d -> b s d"),
            scA[:, :, 32:64])

    nc.compile()
    _PROGRAM_CACHE = nc
    return nc


def _in_maps(context, eps, W1, b1, Wc, Wout, bout):
    import ml_dtypes
    W1T, WoutP, WcT, b1c = _prep_weights(W1, b1, Wc, Wout)
    WoutPb = WoutP.astype(ml_dtypes.bfloat16)
    boutP = _prep_bout(bout)
    ident = np.eye(128, dtype=np.float32)
    maps = []
    for c in range(NCORES):
        maps.append({
            "ctx": np.ascontiguousarray(context[c * BS:(c + 1) * BS]),
            "eps": np.ascontiguousarray(eps[:, c * BS:(c + 1) * BS]),
            "w1t": W1T, "woutpb": WoutPb, "wct": WcT, "b1c": b1c,
            "boutp": boutP, "boutpb": boutP.astype(ml_dtypes.bfloat16),
            "ident": ident,
        })
    return maps


def run(context, eps, W1, b1, Wc, Wout, bout, trace=False):
    context = np.asarray(context, np.float32)
    eps = np.asarray(eps, np.float32)
    W1 = np.asarray(W1, np.float32)
    b1 = np.asarray(b1, np.float32)
    Wc = np.asarray(Wc, np.float32)
    Wout = np.asarray(Wout, np.float32)
    bout = np.asarray(bout, np.float32)
    nc = _build_program()
    maps = _in_maps(context, eps, W1, b1, Wc, Wout, bout)
    res = run_bass_kernel_spmd(nc, maps, core_ids=list(range(NCORES)),
                               trace=trace)
    z = np.empty((S, B, D), np.float32)
    mu = np.empty((S, B, D), np.float32)
    sc = np.empty((S, B, D), np.float32)
    for c in range(NCORES):
        z[:, c * BS:(c + 1) * BS] = res.results[c]["z_out"]
        mu[:, c * BS:(c + 1) * BS] = res.results[c]["mu_out"]
        sc[:, c * BS:(c + 1) * BS] = res.results[c]["sc_out"]
    return (z, mu, sc), res


def kernel(context, eps, W1, b1, Wc, Wout, bout):
    (z, mu, sc), _ = run(context, eps, W1, b1, Wc, Wout, bout)
    return z, mu, sc


# revision 32
# speedup vs baseline: 1.1813x; 1.1064x over previous
"""Trainium2 Bass kernel for nn_AutoRegressiveDistribution (MADE sampling).

Self-contained: hardcodes shapes/sharding. Shards batch B across 8 cores,
runs the D-step autoregressive sampling loop fully on-device per core.

Per-core structure (v2): FOUR independent chains, one per sample s
(width = BS = 128 batch rows each). Per chain and step i the critical
path is:

  hist-MM (PE, float32r, N=256 via free-dim duplication -> 1 cyc/row)
    -> relu (DVE, psum->sbuf, bf16-free)
    -> pair-MM (PE, N=2: only the (mu_i, ps_i) column pair, Wout columns
       are pair-interleaved host-side so the pair is contiguous)
    -> Exp -> Ln(bias=1) (Act, psum->psum softplus)
    -> fused FMA z = eps*sc + mu (GPSIMD scalar_tensor_tensor)
    -> full-state transpose (PE) -> single-row psum->sbuf copy (GPSIMD)
    -> next hist-MM

  The bulk contribution of block i to future column pairs [2i+2, 128)
  is emitted AFTER the FMA so it stays off the critical path; the
  ctx+bias base for the next block is preloaded into PSUM by a
  shifted-identity matmul (also f32r/dup). The loop-invariant
  a_base = Wc @ ctx + b1 is computed chunk-by-chunk interleaved with
  the first ~16 steps so it never stalls the chain.

  float32r bitcasts keep full fp32 precision at 1 cycle/row (vs 4 for
  fp32) for every z-path matmul; only PE-stationary operands and the
  tiny N<=2 pair matmuls stay plain fp32.
"""

import numpy as np
from contextlib import ExitStack

import concourse.bass as bass
import concourse.tile as tile
from concourse import bacc, mybir
from concourse.bass_utils import run_bass_kernel_spmd

D, H, CTX, B, S = 64, 1024, 256, 1024, 4
NCORES = 8
BS = B // NCORES          # 128 batch rows per core
K = S                     # 4 chains per core, one per sample
HP = 2048                 # padded hidden units: block i at [32*(i-1), +cnt[i])
NCH = HP // 128           # a_base unit chunks

FP32 = mybir.dt.float32
F32R = mybir.dt.float32r
BF16 = mybir.dt.bfloat16


def _made_struct():
    mh = (np.arange(H) % (D - 1)) + 1            # degrees 1..63
    perm = np.argsort(mh, kind="stable")
    mh_s = mh[perm]
    cnt = np.bincount(mh_s, minlength=D)          # cnt[d] = #units of degree d
    off = np.concatenate([[0], np.cumsum(cnt)[:-1]]).astype(np.int64)
    return mh, perm, mh_s, cnt, off


def _prep_weights(W1, b1, Wc, Wout):
    """Mask + permute + 32-pad weights host-side (cheap, O(weight size))."""
    mh, perm, mh_s, cnt, off = _made_struct()
    m0 = np.arange(1, D + 1)
    M1 = (mh[:, None] >= m0[None, :]).astype(np.float32)          # (H, D)
    mout = np.concatenate([m0, m0])                                # (2D,)
    Mout = (mout[:, None] > mh[None, :]).astype(np.float32)        # (2D, H)
    W1m = (W1 * M1)[perm]                   # (H, D) permuted rows
    Woutm = (Wout * Mout)[:, perm]          # (2D, H) permuted cols
    src = np.arange(H)
    pdst = 32 * (mh_s - 1) + (src - off[mh_s])   # padded slot of sorted unit
    W1T = np.zeros((D, HP), np.float32)
    W1T[:, pdst] = W1m.T
    WcT = np.zeros((CTX, HP), np.float32)
    WcT[:, pdst] = Wc[perm].T
    b1c = np.zeros((128, NCH), np.float32)
    b1p = np.zeros((HP,), np.float32)
    b1p[pdst] = b1[perm]
    b1c[:, :] = b1p.reshape(NCH, 128).T
    # pair-interleaved output weights: col 2j = mu_j, col 2j+1 = prescale_j
    WoutP = np.zeros((32, D - 1, 2 * D), np.float32)
    mu_rows = Woutm[:D, :]      # (D, H)
    ps_rows = Woutm[D:, :]      # (D, H)
    for j in range(D):
        WoutP[pdst % 32, (mh_s - 1), 2 * j] = mu_rows[j, src]
        WoutP[pdst % 32, (mh_s - 1), 2 * j + 1] = ps_rows[j, src]
    return W1T, WoutP, WcT, b1c


def _prep_bout(bout):
    boutP = np.zeros((128, 2 * D), np.float32)
    boutP[:, 0::2] = bout[:D][None, :]
    boutP[:, 1::2] = bout[D:][None, :]
    return boutP


_PROGRAM_CACHE = None


def _pin_act_table():
    """Make Exp/Ln/Relu resolvable only via natural_log_exp_and_others so
    the act-table chooser doesn't thrash (each LoadActFuncSet ~1.3us)."""
    import concourse.bacc as bacc_mod
    from concourse import hw_specs
    orig = hw_specs.get_activation_tables
    AF = mybir.ActivationFunctionType
    pin = {AF.Exp, AF.Ln, AF.Relu}

    def filtered(arch):
        out = {}
        for name, fns in orig(arch).items():
            if name == "natural_log_exp_and_others":
                out[name] = set(fns)
            else:
                out[name] = set(fns) - pin
        return out

    bacc_mod.get_activation_tables = filtered


def _dup(ap):
    """Duplicate an AP along a broadcast free dim (doubles free size so
    float32r matmuls hit N>=256 and run at 1 cycle/row)."""
    return bass.AP(ap.tensor, ap.offset, [ap.ap[0], [0, 2], ap.ap[-1]])


def _build_program():
    global _PROGRAM_CACHE
    if _PROGRAM_CACHE is not None:
        return _PROGRAM_CACHE
    _pin_act_table()
    _, _, mh_s, cnt, off = _made_struct()

    nc = bacc.Bacc("TRN2", target_bir_lowering=False, debug=False,
                   num_devices=NCORES)

    ctx_d = nc.dram_tensor("ctx", (BS, CTX), FP32, kind="ExternalInput")
    eps_d = nc.dram_tensor("eps", (S, BS, D), FP32, kind="ExternalInput")
    w1t_d = nc.dram_tensor("w1t", (D, HP), F32R, kind="ExternalInput")
    woutpb_d = nc.dram_tensor("woutpb", (32, D - 1, 2 * D), BF16,
                              kind="ExternalInput")
    wct_d = nc.dram_tensor("wct", (CTX, HP), F32R, kind="ExternalInput")
    b1c_d = nc.dram_tensor("b1c", (128, NCH), FP32, kind="ExternalInput")
    boutp_d = nc.dram_tensor("boutp", (128, 2 * D), FP32, kind="ExternalInput")
    boutpb_d = nc.dram_tensor("boutpb", (128, 2 * D), BF16,
                              kind="ExternalInput")
    ident_d = nc.dram_tensor("ident", (128, 128), F32R, kind="ExternalInput")
    z_d = nc.dram_tensor("z_out", (S, BS, D), F32R, kind="ExternalOutput")
    mu_d = nc.dram_tensor("mu_out", (S, BS, D), FP32, kind="ExternalOutput")
    sc_d = nc.dram_tensor("sc_out", (S, BS, D), FP32, kind="ExternalOutput")

    AF = mybir.ActivationFunctionType
    OP = mybir.AluOpType

    with tile.TileContext(nc) as tc, ExitStack() as ctx:
        singles = ctx.enter_context(tc.tile_pool(name="singles", bufs=1))
        abp = ctx.enter_context(tc.tile_pool(name="abp", bufs=2))
        psum = ctx.enter_context(tc.tile_pool(name="psum", bufs=1,
                                              space="PSUM"))

        # ---- input DMAs, priority order ----
        ctx_sb = singles.tile([BS, CTX], FP32)
        nc.sync.dma_start(ctx_sb[:], ctx_d.ap())
        eps_sb = singles.tile([BS, S, D], FP32)
        nc.sync.dma_start(eps_sb[:], eps_d.ap().rearrange("s b d -> b s d"))
        boutp_sb = singles.tile([128, 2 * D], FP32)
        nc.sync.dma_start(boutp_sb[:], boutp_d.ap())
        boutpb_sb = singles.tile([128, 2 * D], BF16)
        nc.sync.dma_start(boutpb_sb[:], boutpb_d.ap())
        ident_sb = singles.tile([128, 128], F32R)
        nc.sync.dma_start(ident_sb[:], ident_d.ap())
        b1c_sb = singles.tile([128, NCH], FP32)
        nc.sync.dma_start(b1c_sb[:], b1c_d.ap())
        wct_sb = singles.tile([128, 2, HP], F32R)
        w1t_sb = singles.tile([D, HP], F32R)
        woutpb_sb = singles.tile([32, D - 1, 2 * D], BF16)
        QH = HP // 4
        for q in range(4):
            nc.sync.dma_start(
                wct_sb[:, :, q * QH:(q + 1) * QH],
                wct_d.ap()[:, q * QH:(q + 1) * QH]
                .rearrange("(k p) h -> p k h", p=128))
            nc.sync.dma_start(w1t_sb[:, q * QH:(q + 1) * QH],
                              w1t_d.ap()[:, q * QH:(q + 1) * QH])
            b0, b1_ = [(0, 16), (16, 32), (32, 48), (48, 63)][q]
            nc.sync.dma_start(woutpb_sb[:, b0:b1_, :],
                              woutpb_d.ap()[:, b0:b1_, :])

        onesb_sb = singles.tile([1, 128], BF16)
        nc.vector.memset(onesb_sb[:], 1.0)


        # ---- PSUM layout, shaped by the HW rule that a start=True matmul
        # marks its whole 2KB bank pending-zero (so a bank can host only one
        # accumulation lifetime at a time):
        #  bank tOUT: all 4 persistent OUT accumulators, seeded by ONE
        #             start=True matmul, then only start=False forever.
        #  bank tZT:  all 4 transpose targets (every write is a fresh
        #             single-matmul start=True group -> safe to share).
        #  banks tPA[c]: per-chain psA (ident start=True -> hist stop,
        #             WAW-ordered, nothing else matmuls this bank) plus the
        #             Act-written scPS strip (engines ignore pending flags).
        #  bank tSET: setup scratch; only single-matmul start=True groups.
        tOUT = psum.tile([128, K, 2 * D], FP32, name="tOUT")
        tZT = psum.tile([D, K, BS], F32R, name="tZT")
        tPA = [psum.tile([128, 512], FP32, tag=f"tPA{c}", name=f"tPA{c}")
               for c in range(K)]
        tSET = psum.tile([128, 512], FP32, name="tSET")
        outP = [tOUT[:, c, :] for c in range(K)]              # (128, 128)
        zTps = [tZT[:, c, :] for c in range(K)]               # (64, 128)
        SC0 = 256                                             # scPS base col

        def aps_ap(c, nn):
            """psA view of tPA[c]: (nn, 2, 128) at cols 0:256."""
            t = tPA[c][0:nn, 0:256]
            return bass.AP(t.tensor, t.offset, [t.ap[0], [128, 2], [1, 128]])

        # ---- ctxT: (BS, CTX) -> (128, 2, BS) via 2 PE transposes ----
        ctxT_sb = singles.tile([128, 2, BS], F32R)
        for kk in range(2):
            ps = tSET[:, kk * BS:kk * BS + BS]
            nc.tensor.transpose(ps, ctx_sb[:, kk * 128:(kk + 1) * 128],
                                ident_sb[:].bitcast(FP32))
            nc.vector.tensor_copy(ctxT_sb[:, kk, :], ps)

        # ---- per-chain state ----
        a_base = singles.tile([128, NCH, 128], F32R)
        z2 = [singles.tile([BS, D], F32R, tag=f"z{c}", name=f"z{c}")
              for c in range(K)]
        muA = singles.tile([BS, K, D], FP32)
        scA = singles.tile([BS, K, D], FP32)
        zT = [singles.tile([D, BS], F32R, tag=f"zT{c}", name=f"zT{c}")
              for c in range(K)]

        for c in range(K):
            nc.vector.memset(z2[c][:].bitcast(FP32), 0.0)

        def a_base_chunk(cc):
            """a_base[:, cc, :] = (WcT chunk).T @ ctxT + b1 chunk.

            In-place accumulation in the tSET bank is safe: every matmul
            writer of this bank is ordered by WAW or data deps, so no
            start=True interloper can land between the two halves."""
            ps = tSET[:, 256:384]
            for kk in range(2):
                nc.tensor.matmul(
                    ps,
                    wct_sb[:, kk, cc * 128:(cc + 1) * 128],
                    ctxT_sb[:, kk, :],
                    start=(kk == 0), stop=(kk == 1))
            nc.vector.tensor_scalar_add(a_base[:, cc, :], ps,
                                        b1c_sb[:, cc:cc + 1])

        a_base_chunk(0)
        a_base_chunk(1)

        def bridge(c, i):
            """z2[c] -> zT[c] row i: full-state PE transpose + 1-row copy."""
            nc.tensor.transpose(zTps[c], z2[c][:], ident_sb[:])
            g = 32 * (i // 32)
            src_rows = tZT[g:g + 32, c, :]
            if c < 2:
                nc.vector.tensor_copy(zT[c][g:g + 32, :], src_rows)
            else:
                nc.scalar.copy(zT[c][g:g + 32, :], src_rows)

        # ---- step 0: bias-only ----
        # one seed matmul covers all 4 OUT accumulators (single start=True
        # lifetime for the whole bank)
        br = boutpb_sb[0:1, :]
        br4 = bass.AP(br.tensor, br.offset, [br.ap[0], [0, K], br.ap[-1]])
        nc.tensor.matmul(tOUT[:, :, :], onesb_sb[:], br4,
                         start=True, stop=False, skip_group_check=True)
        for c in range(K):
            sp = tPA[c][:, SC0 + D:SC0 + D + 1]
            nc.scalar.activation(out=sp, in_=boutp_sb[:, 1:2],
                                 func=AF.Exp, bias=0.0, scale=1.0)
            nc.scalar.activation(out=tPA[c][:, SC0:SC0 + 1], in_=sp,
                                 func=AF.Ln, bias=1.0, scale=1.0)
            nc.vector.scalar_tensor_tensor(
                out=z2[c][:, 0:1], in0=eps_sb[:, c, 0:1],
                scalar=tPA[c][:, SC0:SC0 + 1], in1=boutp_sb[:, 0:1],
                op0=OP.mult, op1=OP.add)
            bridge(c, 0)

        # ---- steps 1..63 ----
        for i in range(1, D):
            nn = int(cnt[i])
            pp = 32 * (i - 1)
            cc, pl = pp // 128, pp % 128
            kk = pl + nn                  # ident rows anchored at 0 so both
                                          # group matmuls share tile pos (0,0)
            if i >= 5 and (i - 5) % 4 == 0:
                nxt = 2 + (i - 5) // 4
                if nxt < NCH:
                    a_base_chunk(nxt)
            # chain-major emission; the tile scheduler handles interleaving.
            # bridge(c) is emitted one chain-block late so copies don't block
            # the next chain's softplus in the Act/DVE queues.
            pending = []
            for c in range(K):
                aps = aps_ap(c, nn)
                nc.tensor.matmul(
                    aps,
                    ident_sb[0:kk, pl:pl + nn],
                    _dup(a_base[0:kk, cc, :]),
                    start=True, stop=False)
                nc.tensor.matmul(
                    aps,
                    w1t_sb[0:i, pp:pp + nn],
                    _dup(zT[c][0:i, :]),
                    start=False, stop=True)
                ab = abp.tile([nn, 128], BF16, tag=f"ab{c}")
                nc.vector.tensor_scalar_max(ab[:], tPA[c][0:nn, 0:128], 0.0)
                nc.tensor.matmul(tOUT[:, c, 2 * i:2 * i + 2],
                                 ab[:],
                                 woutpb_sb[0:nn, i - 1, 2 * i:2 * i + 2],
                                 start=False, stop=(i == D - 1),
                                 skip_group_check=True)
                if i < D - 1:
                    nc.tensor.matmul(tOUT[:, c, 2 * i + 2:2 * D],
                                     ab[:],
                                     woutpb_sb[0:nn, i - 1, 2 * i + 2:],
                                     start=False, stop=False,
                                     skip_group_check=True)
                sp = tPA[c][:, SC0 + D + (i % 2):SC0 + D + (i % 2) + 1]
                nc.scalar.activation(out=sp,
                                     in_=tOUT[:, c, 2 * i + 1:2 * i + 2],
                                     func=AF.Exp, bias=0.0, scale=1.0)
                nc.scalar.activation(out=tPA[c][:, SC0 + i:SC0 + i + 1],
                                     in_=sp, func=AF.Ln, bias=1.0, scale=1.0)
                nc.vector.scalar_tensor_tensor(
                    out=z2[c][:, i:i + 1], in0=eps_sb[:, c, i:i + 1],
                    scalar=tPA[c][:, SC0 + i:SC0 + i + 1],
                    in1=tOUT[:, c, 2 * i:2 * i + 1],
                    op0=OP.mult, op1=OP.add)
                if pending:
                    bridge(pending.pop(), i)
                if i < D - 1:
                    pending.append(c)

            if pending:
                bridge(pending.pop(), i)

        # ---- tail: extract mu (even cols) + sc, DMA out ----
        for c in range(K):
            sl = tOUT[:, c, :]
            mu_ap = bass.AP(sl.tensor, sl.offset, [sl.ap[0], [2, D]])
            nc.vector.tensor_copy(muA[:, c, :], mu_ap)
            nc.scalar.copy(scA[:, c, :], tPA[c][:, SC0:SC0 + D])
            nc.sync.dma_start(z_d.ap()[c], z2[c][:])
        nc.sync.dma_start(mu_d.ap().rearrange("s b d -> b s d"), muA[:])
        nc.sync.dma_start(sc_d.ap().rearrange("s b d -> b s d"), scA[:])

    nc.compile()
    _PROGRAM_CACHE = nc
    return nc


def _in_maps(context, eps, W1, b1, Wc, Wout, bout):
    import ml_dtypes
    W1T, WoutP, WcT, b1c = _prep_weights(W1, b1, Wc, Wout)
    WoutPb = WoutP.astype(ml_dtypes.bfloat16)
    boutP = _prep_bout(bout)
    ident = np.eye(128, dtype=np.float32)
    maps = []
    for c in range(NCORES):
        maps.append({
            "ctx": np.ascontiguousarray(context[c * BS:(c + 1) * BS]),
            "eps": np.ascontiguousarray(eps[:, c * BS:(c + 1) * BS]),
            "w1t": W1T, "woutpb": WoutPb, "wct": WcT, "b1c": b1c,
            "boutp": boutP, "boutpb": boutP.astype(ml_dtypes.bfloat16),
            "ident": ident,
        })
    return maps


def run(context, eps, W1, b1, Wc, Wout, bout, trace=False):
    context = np.asarray(context, np.float32)
    eps = np.asarray(eps, np.float32)
    W1 = np.asarray(W1, np.float32)
    b1 = np.asarray(b1, np.float32)
    Wc = np.asarray(Wc, np.float32)
    Wout = np.asarray(Wout, np.float32)
    bout = np.asarray(bout, np.float32)
    nc = _build_program()
    maps = _in_maps(context, eps, W1, b1, Wc, Wout, bout)
    res = run_bass_kernel_spmd(nc, maps, core_ids=list(range(NCORES)),
                               trace=trace)
    z = np.empty((S, B, D), np.float32)
    mu = np.empty((S, B, D), np.float32)
    sc = np.empty((S, B, D), np.float32)
    for c in range(NCORES):
        z[:, c * BS:(c + 1) * BS] = res.results[c]["z_out"]
        mu[:, c * BS:(c + 1) * BS] = res.results[c]["mu_out"]
        sc[:, c * BS:(c + 1) * BS] = res.results[c]["sc_out"]
    return (z, mu, sc), res


def kernel(context, eps, W1, b1, Wc, Wout, bout):
    (z, mu, sc), _ = run(context, eps, W1, b1, Wc, Wout, bout)
    return z, mu, sc
